# revision 2
# baseline (speedup 1.0000x reference)
"""GCN layer (normalized adjacency @ features -> linear -> relu) on 8 TRN2 NeuronCores.

Strategy (row-sharded, 1D node partition):
  - Host shards adj by rows (P=1024 rows/core) and adds the identity diagonal
    into each shard so the SPMD program is uniform across cores.
  - Phase 1 (per core): stream the [P, N] f32 shard from HBM in bands, convert
    to bf16, transpose on the PE (matmul-by-identity) into an SBUF-resident
    [N, P]-layout bf16 copy (16MB; SBUF holds it, so adj is read from HBM once).
    Row sums accumulate on the PE via ones-vector matmuls over the transposed
    tiles. Features stream + convert to bf16 in parallel.
  - AllGather the per-core row sums ([1,P] f32 -> [8,P]); d = rsqrt(rowsum)
    via Sqrt + reciprocal + one Newton step.
  - Phase 2: DF = d * features (bf16); out_pre.T = sum_t DF[t].T-contraction
    with the resident transposed adj tiles (128 accumulating bf16 matmuls);
    linear via a small fp32 matmul with W.T; scale rows by d_own, add bias,
    relu; DMA out. Host concatenates the 8 [P,128] outputs.
"""

import numpy as np
import ml_dtypes

import concourse.bass as bass
import concourse.bacc as bacc
import concourse.mybir as mybir
import concourse.tile as tile
from concourse.bass_utils import run_bass_kernel_spmd

F32 = mybir.dt.float32
BF16 = mybir.dt.bfloat16

N_FULL = 8192
F_DIM = 128
NUM_CORES = 8


def build_kernel(P=1024, N=8192, F=128, num_cores=8):
    """Build the SPMD Bass program. P = rows per core; N = total nodes."""
    assert P % 128 == 0 and N % 256 == 0 and F == 128
    n_st = P // 128          # row stripes per core
    n_jb = N // 128          # j-blocks (transposed tiles); must be <= 128
    assert n_jb <= 128
    QCOL = 256               # adj columns consumed per band
    n_q = N // QCOL
    CH = min(512, P)         # rowsum psum chunk
    n_half = P // CH
    FCH = 1024 if N % 1024 == 0 else 128 * min(8, n_jb)  # feat rows per staging chunk
    n_u = N // FCH
    ftb = FCH // 128         # j-blocks per feat chunk

    nc = bacc.Bacc("TRN2", target_bir_lowering=False, debug=False,
                   num_devices=num_cores)

    adj_h = nc.declare_dram_parameter("adj_s", [P, N], F32, isOutput=False)
    feat_h = nc.declare_dram_parameter("feat", [N, F], F32, isOutput=False)
    w_h = nc.declare_dram_parameter("w", [F, F], F32, isOutput=False)
    bias_h = nc.declare_dram_parameter("bias_b", [128, F], F32, isOutput=False)
    eye16_h = nc.declare_dram_parameter("eye16", [128, 128], BF16, isOutput=False)
    eye32_h = nc.declare_dram_parameter("eye32", [128, 128], F32, isOutput=False)
    ones16_h = nc.declare_dram_parameter("ones16", [128, 1], BF16, isOutput=False)
    out_h = nc.declare_dram_parameter("out", [P, F], F32, isOutput=True)

    r_local = nc.dram_tensor("r_local", [1, P], F32)
    r_full = nc.dram_tensor("r_full", [num_cores, P], F32, addr_space="Shared")

    # DRAM access patterns
    adj_ap = adj_h.ap().rearrange("(s p) (q j) -> p s q j", p=128, j=QCOL)
    feat_ap = feat_h.ap().rearrange("(u t p) f -> u p t f", t=ftb, p=128)
    out_ap = out_h.ap().rearrange("(s p) f -> p s f", p=128)

    with tile.TileContext(nc) as tc:
        with tc.tile_pool(name="const", bufs=1) as cpool, \
             tc.tile_pool(name="atp", bufs=n_jb) as atp:

            eye16 = cpool.tile([128, 128], BF16)
            nc.sync.dma_start(eye16, eye16_h[:])
            eye32 = cpool.tile([128, 128], F32)
            nc.sync.dma_start(eye32, eye32_h[:])
            ones16 = cpool.tile([128, 1], BF16)
            nc.sync.dma_start(ones16, ones16_h[:])
            w_sb = cpool.tile([128, F], F32)
            nc.sync.dma_start(w_sb, w_h[:])
            bias_bc = cpool.tile([128, F], F32)
            nc.sync.dma_start(bias_bc, bias_h[:])
            feat16 = cpool.tile([128, n_jb, F], BF16)
            rs_sb = cpool.tile([1, P], F32)

            at_tiles = []

            with tc.tile_pool(name="ph1", bufs=2) as p1, \
                 tc.tile_pool(name="ps1", bufs=1, space="PSUM") as ps1:

                # rowsum accumulators, pinned across phase 1
                pr = [ps1.tile([1, CH], F32, tag=f"pr{h}", name=f"pr{h}")
                      for h in range(n_half)]

                # feature load + bf16 convert (overlaps the adj stream)
                for u in range(n_u):
                    fstage = p1.tile([128, ftb, F], F32, tag="fstage")
                    nc.sync.dma_start(fstage, feat_ap[u])
                    nc.vector.tensor_copy(feat16[:, u * ftb:(u + 1) * ftb, :], fstage)

                for q in range(n_q):
                    band = p1.tile([128, n_st, QCOL], F32, tag="band")
                    nc.sync.dma_start(band, adj_ap[:, :, q])
                    band16 = p1.tile([128, n_st, QCOL], BF16, tag="band16")
                    nc.vector.tensor_copy(band16, band)
                    for tq in range(QCOL // 128):
                        tg = q * (QCOL // 128) + tq
                        pt = ps1.tile([128, P], F32, tag="pt", bufs=2, name="pt")
                        for s in range(n_st):
                            nc.tensor.matmul(
                                pt[:, s * 128:(s + 1) * 128],
                                lhsT=band16[:, s, tq * 128:(tq + 1) * 128],
                                rhs=eye16,
                                start=True, stop=True,
                            )
                        a_t = atp.tile([128, P], BF16, tag="a_t", name="a_t")
                        nc.scalar.copy(a_t, pt)
                        at_tiles.append(a_t)
                        for h in range(n_half):
                            nc.tensor.matmul(
                                pr[h],
                                lhsT=ones16,
                                rhs=a_t[:, h * CH:(h + 1) * CH],
                                start=(tg == 0), stop=(tg == n_jb - 1),
                            )

                for h in range(n_half):
                    nc.vector.tensor_copy(rs_sb[0:1, h * CH:(h + 1) * CH], pr[h])

            nc.sync.dma_start(r_local[:], rs_sb)
            nc.gpsimd.collective_compute(
                "AllGather", mybir.AluOpType.bypass,
                replica_groups=[list(range(num_cores))],
                ins=[r_local[:].opt()],
                outs=[r_full[:].opt()],
            )

            with tc.tile_pool(name="ph2", bufs=1) as p2, \
                 tc.tile_pool(name="ps2", bufs=1, space="PSUM") as ps2:

                # d for all N columns: load gathered rowsums as [n_jb, 128],
                # PE-transpose to [128, n_jb], then rsqrt with Newton refine.
                rf_sb = p2.tile([n_jb, 128], F32)
                nc.sync.dma_start(rf_sb, r_full.ap().rearrange(
                    "c (s p) -> (c s) p", p=128))
                prT = ps2.tile([128, n_jb], F32, tag="prT")
                nc.tensor.matmul(prT, lhsT=rf_sb, rhs=eye32[0:n_jb, 0:n_jb],
                                 start=True, stop=True)

                # d for own rows from the local rowsums: [n_st,128] -> [128,n_st]
                rlo_sb = p2.tile([n_st, 128], F32)
                nc.sync.dma_start(rlo_sb, r_local.ap().rearrange(
                    "o (s p) -> (o s) p", p=128))
                prO = ps2.tile([128, n_st], F32, tag="prO")
                nc.tensor.matmul(prO, lhsT=rlo_sb, rhs=eye32[0:n_st, 0:n_st],
                                 start=True, stop=True)

                def rsqrt_newton(psum_r, width, nm):
                    sq = p2.tile([128, width], F32, tag=f"sq{nm}", name=f"sq{nm}")
                    nc.scalar.activation(sq, psum_r,
                                         mybir.ActivationFunctionType.Sqrt)
                    y0 = p2.tile([128, width], F32, tag=f"y0{nm}", name=f"y0{nm}")
                    nc.vector.reciprocal(y0, sq)
                    yy = p2.tile([128, width], F32, tag=f"yy{nm}", name=f"yy{nm}")
                    nc.vector.tensor_mul(yy, y0, y0)
                    ryy = p2.tile([128, width], F32, tag=f"ry{nm}", name=f"ryy{nm}")
                    nc.vector.tensor_mul(ryy, yy, psum_r)
                    corr = p2.tile([128, width], F32, tag=f"co{nm}", name=f"corr{nm}")
                    nc.vector.tensor_scalar(out=corr, in0=ryy, scalar1=-0.5,
                                            scalar2=1.5,
                                            op0=mybir.AluOpType.mult,
                                            op1=mybir.AluOpType.add)
                    d = p2.tile([128, width], F32, tag=f"d{nm}", name=f"d{nm}")
                    nc.vector.tensor_mul(d, y0, corr)
                    return d

                d_all = rsqrt_newton(prT, n_jb, "a")
                d_own = rsqrt_newton(prO, n_st, "o")

                # DF tiles (bf16), one per j-block
                df_tiles = []
                for t in range(n_jb):
                    df = p2.tile([128, F], BF16, tag="df", bufs=n_jb, name="df")
                    nc.vector.tensor_scalar(out=df, in0=feat16[:, t, :],
                                            scalar1=d_all[:, t:t + 1],
                                            scalar2=None,
                                            op0=mybir.AluOpType.mult)
                    df_tiles.append(df)

                # main matmul: out_pre.T [F, P] accumulated over j-blocks
                CH2 = min(512, P)
                n_h2 = P // CH2
                pm = [ps2.tile([128, CH2], F32, tag=f"pm{h}", name=f"pm{h}")
                      for h in range(n_h2)]
                for t in range(n_jb):
                    for h in range(n_h2):
                        nc.tensor.matmul(
                            pm[h],
                            lhsT=df_tiles[t],
                            rhs=at_tiles[t][:, h * CH2:(h + 1) * CH2],
                            start=(t == 0), stop=(t == n_jb - 1),
                        )
                opre = p2.tile([128, P], F32)
                for h in range(n_h2):
                    nc.scalar.copy(opre[:, h * CH2:(h + 1) * CH2], pm[h])

                # W.T via PE transpose
                pw = ps2.tile([128, F], F32, tag="pw")
                nc.tensor.matmul(pw, lhsT=w_sb, rhs=eye32, start=True, stop=True)
                wt_sb = p2.tile([128, F], F32)
                nc.scalar.copy(wt_sb, pw)

                out_sb = p2.tile([128, n_st, F], F32)
                for s in range(n_st):
                    p2m = ps2.tile([128, F], F32, tag="p2m", bufs=2, name="p2m")
                    nc.tensor.matmul(p2m, lhsT=opre[:, s * 128:(s + 1) * 128],
                                     rhs=wt_sb, start=True, stop=True)
                    epi = p2.tile([128, F], F32, tag="epi", bufs=2, name="epi")
                    nc.vector.scalar_tensor_tensor(
                        out=epi, in0=p2m, scalar=d_own[:, s:s + 1], in1=bias_bc,
                        op0=mybir.AluOpType.mult, op1=mybir.AluOpType.add)
                    nc.vector.tensor_scalar_max(out_sb[:, s, :], epi, 0.0)

                nc.sync.dma_start(out_ap, out_sb)

    nc.compile()
    return nc


def make_in_maps(adj, features, W, b, P, num_cores):
    """Shard inputs; adds the +I diagonal into each adj row-shard."""
    adj = np.asarray(adj, dtype=np.float32)
    features = np.asarray(features, dtype=np.float32)
    W = np.asarray(W, dtype=np.float32)
    b = np.asarray(b, dtype=np.float32)
    eye16 = np.eye(128, dtype=ml_dtypes.bfloat16)
    eye32 = np.eye(128, dtype=np.float32)
    ones16 = np.ones((128, 1), dtype=ml_dtypes.bfloat16)
    bias_b = np.broadcast_to(b[None, :], (128, b.shape[0])).copy()
    in_maps = []
    idx = np.arange(P)
    for c in range(num_cores):
        sh = adj[c * P:(c + 1) * P, :].copy()
        sh[idx, c * P + idx] += 1.0
        in_maps.append({
            "adj_s": sh,
            "feat": features,
            "w": W,
            "bias_b": bias_b,
            "eye16": eye16,
            "eye32": eye32,
            "ones16": ones16,
        })
    return in_maps


_NC_CACHE = {}


def get_nc(P=N_FULL // NUM_CORES, N=N_FULL, F=F_DIM, num_cores=NUM_CORES):
    key = (P, N, F, num_cores)
    if key not in _NC_CACHE:
        _NC_CACHE[key] = build_kernel(P, N, F, num_cores)
    return _NC_CACHE[key]


def kernel(**inputs):
    adj = np.asarray(inputs["adj"], dtype=np.float32)
    features = np.asarray(inputs["features"], dtype=np.float32)
    W = np.asarray(inputs["W"], dtype=np.float32)
    b = np.asarray(inputs["b"], dtype=np.float32)
    n = adj.shape[0]
    P = n // NUM_CORES
    nc = get_nc(P, n, features.shape[1], NUM_CORES)
    in_maps = make_in_maps(adj, features, W, b, P, NUM_CORES)
    res = run_bass_kernel_spmd(nc, in_maps, core_ids=list(range(NUM_CORES)))
    outs = [np.asarray(res.results[c]["out"], dtype=np.float32)
            for c in range(NUM_CORES)]
    return np.concatenate(outs, axis=0)
